# revision 18
# baseline (speedup 1.0000x reference)
"""Multi-head channel-attention kernel for Trainium2 (8 NeuronCores, SPMD).

Reference computation (per batch b, x = [256, N] with N = 64*64 = 4096):
    qkv   = w_qkv @ x
    q,k,v = per-head [256, N] slices of qkv
    logit = (q*scale) @ k.T          # [256, 256] (contraction over N)
    wts   = softmax(logit, -1)
    out_h = wts @ v
    y     = w_out @ stack_h(out_h) + b_out

Distribution: pure data-parallel — batch 8 across 8 cores, one batch per
core, no collectives.

The kernel exploits that attention is over the *channel* axis (n >> c):

    logit_h = (Wq_h * scale) @ (x @ x.T) @ Wk_h.T
    y       = (sum_h W_h @ softmax_h @ Wv_h) @ x + b  =  Wstar @ x + b

so the only n-wide work is the Gram matrix G = x @ x.T (one pass over x)
and the final Wstar @ x (second pass). Everything else is [256,256]-sized.

Schedule (v2; all matmuls TensorE, bf16 operands, fp32 PSUM):
  - Input DMAs are packed into 8 descriptors; the two leading xT groups
    are *hoisted before the startup all-engine barrier* so their data is
    already landing while the runtime preamble runs.  ~18 junk warmup
    matmuls are likewise hoisted pre-barrier so the PE HAM clock-gate is
    released (2.4 GHz) by the time G starts.
  - A dummy exp right after the triggers preloads the ACT spline table
    (1.3 us) off the critical path.
  - G exploits symmetry: strip0 = G[0:128,:] fully, G11 = G[128:,128:]
    by matmul, G10 = G01^T by one PE-transpose against a shipped identity.
  - Per-head [256,256] stages run A(batched head-pairs, free dim 512) ->
    L -> softmax -> M -> Wstar, software-pipelined in emission order
    A01 A23 L0 L1 L2 L3 M0 M1 W0 M2 W1 M3 W2 W3 across 8 PSUM banks so
    the PE never waits on the ACT softmax chain.
  - y = WstarT.T @ x + b is drained straight to bf16 (output is bf16,
    upcast on host) and stored in 5 tapered groups per half so the final
    store is only 2x128 KB.
"""

import numpy as np
import ml_dtypes

import concourse.bass as bass
import concourse.mybir as mybir
import concourse.tile as tile
from concourse.bass import ts
from concourse.bass_utils import run_bass_kernel_spmd
from concourse.vector_clock import ScopedClock

B, DIM, H, W = 8, 256, 64, 64
HEADS = 4
N = H * W            # 4096
P = 128
KT = DIM // P        # 2 channel tiles
NT = N // P          # 32 n-tiles of 128
NG = 4               # xT shipped in 4 groups of 2 slabs (slab = 4 n-tiles)
NCH = N // 512       # 8 n-chunks of 512
N_CORES = 8
WCOL = 8 * HEADS * DIM + P       # w_all free size: wkq0|wkq1|wvo0|wvo1|eye = 8320

F32 = mybir.dt.float32
BF16 = mybir.dt.bfloat16
FP8 = mybir.dt.float8e3            # TRN E3M4: 4-bit mantissa, range +-15.5
NPBF16 = ml_dtypes.bfloat16
NPFP8 = ml_dtypes.float8_e3m4


def _split_multi_waits(nc, max_waits=1):
    """The walrus build in this container rejects instructions carrying more
    than one sync-wait. Move excess waits onto same-engine carrier NOPs
    inserted immediately before the instruction (engines are in-order, so
    waiting earlier on the same stream is equivalent)."""
    n_split = 0
    for f in nc.m.functions:
        for bb in f.blocks:
            old = list(bb.instructions)
            new = []
            changed = False
            for inst in old:
                si = inst.sync_info
                waits = list(si.on_wait) if si and si.on_wait else []
                if len(waits) > max_waits:
                    changed = True
                    for w in waits[max_waits:]:
                        n_split += 1
                        new.append(
                            mybir.InstNoOp(
                                name=f"wsplit_{n_split}_{inst.name}",
                                engine=inst.engine,
                                ins=[],
                                outs=[],
                                sync_info=mybir.SyncInfo(on_wait=[w], on_update=[]),
                            )
                        )
                    inst.sync_info = mybir.SyncInfo(
                        on_wait=waits[:max_waits], on_update=si.on_update
                    )
                new.append(inst)
            if changed:
                bb.instructions = new
    return n_split


def _minimal_exit(self, tick_clock, wait_clock):
    """TileContext._drain_and_barrier replacement: one SP drain carrying the
    global-clock waits (split onto NOPs by _split_multi_waits afterwards).

    The stock exit adds two all-engine barriers and ~200 per-semaphore
    clears (~10 us). They are redundant here: the bass preamble range-clears
    the whole kernel semaphore range at startup, and bass's own postamble
    still drains every engine.
    """
    nc = self.nc
    drain = nc.sync.drain()
    wait_clock.add_sem_waits(drain.ins, ScopedClock({None: tick_clock.global_clock}))
    popped = nc._tile_sem_poison_stack.pop()
    assert popped is self._sem_poison


def _hoist_to_main(nc):
    """Move dependency-free startup work (leading input-DMA triggers and PE
    warmup matmuls) from the tile block into `main`, before the startup
    all-engine barrier. The engines finish their runtime preambles ~5-6 us
    in; hoisted work overlaps the remaining skew so the first xT bytes are
    landing and the PE clock-gate is released when the kernel body starts.
    Semaphores are cleared by the runtime at load, so pre-barrier sem
    increments are observed correctly by post-barrier waiters."""
    names = set(getattr(nc, "_hoist_names", []))
    if not names:
        return
    f = nc.m.functions[0]
    main = f.blocks[0]
    moved = []
    for b in f.blocks[1:]:
        old = list(b.instructions)
        got = [i for i in old if i.name in names]
        if got:
            b.instructions = [i for i in old if i.name not in names]
            moved.extend(got)
    # safety: a hoisted inst may only wait on sems updated by other hoisted
    # insts (sems are runtime-cleared at load, so pre-barrier inc/wait pairs
    # among hoisted insts resolve correctly).
    updated = {
        u.ant_name
        for i in moved
        if i.sync_info
        for u in (i.sync_info.on_update or [])
    }
    for i in moved:
        si = i.sync_info
        for w in (si.on_wait or []) if si else []:
            assert w.ant_name in updated, (
                f"hoisted inst {i.name} waits on foreign sem {w.ant_name}"
            )
    main_list = list(main.instructions)
    # insert after the const-AP memsets, before the startup barrier drains
    pos = next(
        (
            idx
            for idx, i in enumerate(main_list)
            if i.sync_info
            and any("barrier" in (w.ant_name or "") for w in (i.sync_info.on_wait or []))
        ),
        len(main_list),
    )
    main.instructions = main_list[:pos] + moved + main_list[pos:]


def build_program():
    """Build the single-core Bass program (run SPMD across 8 cores)."""
    nc = bass.Bass()

    # xt group g: [128, 2, 4, 256]; element (p, s, a, c) = x.T[(2g+s)*1024 + a*128 + p, c]
    xt_d = [
        nc.declare_dram_parameter(f"xt{g}", [P, 2, NT // (2 * NG), DIM], FP8, isOutput=False)
        for g in range(NG)
    ]
    # w_all: [128, 8320] = [wqT|wkT (k=0) | wqT|wkT (k=1) | wv|woT (k=0) | wv|woT (k=1) | eye128]
    w_d = nc.declare_dram_parameter("w", [P, WCOL], BF16, isOutput=False)
    x_d = nc.declare_dram_parameter("x", [P, KT, N], BF16, isOutput=False)
    b_d = nc.declare_dram_parameter("b", [P, KT], F32, isOutput=False)
    y_d = nc.declare_dram_parameter("y", [P, KT, N], BF16, isOutput=True)

    prev_exit = tile.TileContext._drain_and_barrier
    tile.TileContext._drain_and_barrier = _minimal_exit
    try:
        _build_body(nc, tc_args=(xt_d, w_d, x_d, b_d, y_d))
    finally:
        tile.TileContext._drain_and_barrier = prev_exit

    _hoist_to_main(nc)
    _split_multi_waits(nc)
    return nc


def _build_body(nc, tc_args):
    xt_d, w_d, x_d, b_d, y_d = tc_args
    OQT, OKT, OV, OOT = 0, HEADS * DIM, 0, HEADS * DIM
    SCALE = float(DIM) ** -0.5

    with tile.TileContext(nc) as tc:
        with (
            tc.tile_pool(name="wpool", bufs=1) as wpool,
            tc.tile_pool(name="spool", bufs=2) as spool,
            tc.tile_pool(name="ypool", bufs=2) as ypool,
            tc.tile_pool(name="psum", bufs=1, space="PSUM") as psum,
        ):
            hoist = []
            junk = wpool.tile([P, P], BF16, tag="junk")
            hoist.append(nc.gpsimd.memset(junk[:], 0).ins.name)

            # ---- input loads. First xT slab is tiny (256 KB) so G's first
            # operand lands ~2 us after the startup barrier; the rest stream
            # on both HWDGE rings just ahead of G's consumption.  Weight
            # halves are split so eye/wkq (needed right after G) beat wvo.
            xt_sb = []
            for g in range(NG):
                t = wpool.tile([P, 2, NT // (2 * NG), DIM], FP8, tag=f"xt{g}")
                xt_sb.append(t)
            w_sb = wpool.tile([P, WCOL], BF16, tag="w")
            x_sb = wpool.tile([P, KT, N], BF16, tag="x")
            b_sb = wpool.tile([P, KT], F32, tag="b")

            HD = HEADS * DIM        # 1024
            HD2 = 2 * HEADS * DIM   # 2048
            # scalar's ring (q10) starts ~1 us earlier: it carries the two
            # leading xT slabs, then wkT|eye (needed right after G), wvo and
            # the second x half.  sync (q1) carries xT groups 1-3, wqT, the
            # first x half, b and later the y stores.
            hoist.append(nc.scalar.dma_start(xt_sb[0][:, 0], xt_d[0][:, 0]).ins.name)
            hoist.append(nc.scalar.dma_start(xt_sb[0][:, 1], xt_d[0][:, 1]).ins.name)
            hoist.append(nc.sync.dma_start(xt_sb[1][:], xt_d[1][:]).ins.name)
            hoist.append(nc.scalar.dma_start(xt_sb[2][:], xt_d[2][:]).ins.name)
            nc.sync.dma_start(xt_sb[3][:], xt_d[3][:])
            nc.scalar.dma_start(w_sb[:, 0 : 2 * HD + P], w_d[:, 0 : 2 * HD + P])
            nc.sync.dma_start(
                w_sb[:, 2 * HD + P : 4 * HD + P], w_d[:, 2 * HD + P : 4 * HD + P]
            )
            nc.sync.dma_start(x_sb[:, :, 0 : N // 2], x_d[:, :, 0 : N // 2])
            nc.sync.dma_start(b_sb[:], b_d[:])

            def wkT(k):   # [128, 1024] = wkT rows k*128:(k+1)*128
                return w_sb[:, k * HD : (k + 1) * HD]

            eye = w_sb[:, 2 * HD : 2 * HD + P]

            def wqT(k):   # [128, 1024] = wqT rows k*128:(k+1)*128
                return w_sb[:, 2 * HD + P + k * HD : 2 * HD + P + (k + 1) * HD]

            def wvo(k):   # [128, 2048] = [wv | woT] rows k*128:(k+1)*128
                return w_sb[:, 4 * HD + P + k * HD2 : 4 * HD + P + (k + 1) * HD2]

            # ---- dummy exp: preload the ACT spline table off-path --------
            dummy = spool.tile([P, 1], F32, tag="dummy", bufs=1)
            nc.scalar.activation(dummy[:], junk[:, 0:1], mybir.ActivationFunctionType.Exp)
            # wvo + second x half queue behind wkT on ACT's ring
            nc.scalar.dma_start(w_sb[:, 4 * HD + P : WCOL], w_d[:, 4 * HD + P : WCOL])
            nc.scalar.dma_start(x_sb[:, :, N // 2 : N], x_d[:, :, N // 2 : N])

            # ---- PE warmup: junk matmuls during the xT DMA latency window
            # release the HAM clock-gate so G runs at 2.4 GHz (results
            # discarded; bank g1 is re-cleared by G's start=True).
            wps = psum.tile([P, DIM], F32, tag="g1", bufs=1, name="warmps")
            for wi in range(18):
                hoist.append(
                    nc.tensor.matmul(
                        wps[:, 0:P], junk[:], junk[:], start=True, stop=True
                    ).ins.name
                )
            nc._hoist_names = hoist
            # post-barrier continuation: keep the PE busy (HAM warm) while
            # the first xT bytes are still in flight
            wps2 = psum.tile([P, DIM], F32, tag="g1", bufs=1, name="warmps2")
            for wi in range(20):
                nc.tensor.matmul(wps2[:, 0:P], junk[:], junk[:], start=True, stop=True)

            # ---- G = x @ x.T (fp32 PSUM, symmetric: strip0 + G11 + G01^T) -
            g_ps = []
            for ct in range(KT):
                gp = psum.tile([P, DIM], F32, tag=f"g{ct}", bufs=1)
                g_ps.append(gp)
            for i in range(NT):
                g, rem = divmod(i, NT // NG)
                s, a = divmod(rem, NT // (2 * NG))
                xa = xt_sb[g][:, s, a, :]
                nc.tensor.matmul(
                    g_ps[0][:], xa[:, ts(0, P)], xa,
                    start=(i == 0), stop=(i == NT - 1),
                )
                nc.tensor.matmul(
                    g_ps[1][:, ts(1, P)], xa[:, ts(1, P)], xa[:, ts(1, P)],
                    start=(i == 0), stop=(i == NT - 1),
                )
            g_sb = []
            for ct in range(KT):
                g = spool.tile([P, DIM], BF16, tag=f"gs{ct}", bufs=1, name=f"g{ct}")
                g_sb.append(g)
            nc.vector.tensor_copy(g_sb[0][:, ts(0, P)], g_ps[0][:, ts(0, P)])
            nc.scalar.copy(g_sb[0][:, ts(1, P)], g_ps[0][:, ts(1, P)])
            nc.vector.tensor_copy(g_sb[1][:, ts(1, P)], g_ps[1][:, ts(1, P)])
            # G10 = G01^T via PE transpose (lhsT=G01 bf16, rhs=identity)
            tp = psum.tile([P, DIM], F32, tag="g0", bufs=1, name="tpose")
            nc.tensor.matmul(tp[:, 0:P], g_sb[0][:, ts(1, P)], eye, start=True, stop=True)
            nc.vector.tensor_copy(g_sb[1][:, ts(0, P)], tp[:, 0:P])

            # ---- per-head stages, software-pipelined across heads --------
            at2 = {}     # (hp, ct) -> [128, 512] bf16 (A for head pair hp)
            es_all = {}  # h -> [e_ct0, e_ct1]
            m_sb = {}

            def stage_A(hp):
                aps = [
                    psum.tile([P, 2 * DIM], F32, tag="am", bufs=2, name=f"ap{hp}_{ct}")
                    for ct in range(KT)
                ]
                for k in range(KT):
                    for ct in range(KT):
                        # A[c', d(2 heads)] = sum_c'' G[c'', c'] wkT[c'', d]
                        nc.tensor.matmul(
                            aps[ct][:],
                            g_sb[k][:, ts(ct, P)],
                            wkT(k)[:, hp * 2 * DIM : (hp + 1) * 2 * DIM],
                            start=(k == 0),
                            stop=(k == KT - 1),
                        )
                for ct in range(KT):
                    at = spool.tile([P, 2 * DIM], BF16, tag=f"a{ct}", name=f"at{hp}_{ct}")
                    if ct == 0:
                        nc.vector.tensor_copy(at[:], aps[ct][:])
                    else:
                        nc.scalar.copy(at[:], aps[ct][:])
                    at2[(hp, ct)] = at

            def stage_L(h):
                # both c-tile halves of L go into ONE psum bank ([128, 2, 256]
                # fp32 = 2 KB/partition): the c1 half starts with start=False
                # and overwrites via the has_written bits cleared by c0's
                # start=True, so one batched exp covers the whole head.
                lp = psum.tile([P, KT, DIM], F32, tag="l", bufs=2, name=f"lp{h}")
                for ct in range(KT):
                    for k in range(KT):
                        # L[c, d] = sum_c' wqT[c', c] A[c', d]
                        nc.tensor.matmul(
                            lp[:, ct, :],
                            wqT(k)[:, h * DIM + ct * P : h * DIM + (ct + 1) * P],
                            at2[(h // 2, k)][:, (h % 2) * DIM : (h % 2 + 1) * DIM],
                            start=(ct == 0 and k == 0),
                            stop=(ct == KT - 1 and k == KT - 1),
                        )
                # softmax (ACT exp; DVE row-sums/recip; normalize split DVE/ACT)
                e = spool.tile([P, KT, DIM], BF16, tag="e", bufs=2, name=f"e{h}")
                sm = spool.tile([P, KT], F32, tag="s", bufs=2, name=f"s{h}")
                r = spool.tile([P, KT], F32, tag="r", bufs=2, name=f"r{h}")
                nc.scalar.activation(
                    e[:], lp[:], mybir.ActivationFunctionType.Exp,
                )
                for ct in range(KT):
                    nc.vector.reduce_sum(
                        sm[:, ct : ct + 1], e[:, ct, :], axis=mybir.AxisListType.X
                    )
                nc.vector.reciprocal(r[:], sm[:])
                nc.vector.tensor_scalar_mul(e[:, 0, :], e[:, 0, :], r[:, 0:1])
                # exp output is positive, so Relu(r * e) == r * e on ACT
                nc.scalar.activation(
                    e[:, 1, :], e[:, 1, :], mybir.ActivationFunctionType.Relu,
                    scale=r[:, 1:2],
                )
                es_all[h] = e

            def stage_M(h):
                es = es_all[h]
                for dt2 in range(KT):
                    pm = psum.tile([P, DIM], F32, tag="am", bufs=2, name=f"pm{h}_{dt2}")
                    for ct in range(KT):
                        # M_hT[d, o] = sum_c Ehat[c, d] woT[c, o]
                        nc.tensor.matmul(
                            pm[:],
                            es[:, ct, ts(dt2, P)],
                            wvo(ct)[:, OOT + h * DIM : OOT + (h + 1) * DIM],
                            start=(ct == 0),
                            stop=(ct == KT - 1),
                        )
                    mt = spool.tile([P, DIM], BF16, tag=f"m{h}_{dt2}", bufs=1,
                                    name=f"mt{h}_{dt2}")
                    m_sb[(h, dt2)] = mt
                    if dt2 == 0:
                        nc.vector.tensor_copy(mt[:], pm[:])
                    else:
                        nc.scalar.copy(mt[:], pm[:])

            # WstarT accumulates across all heads into banks g0/g1
            wst_ps = [
                psum.tile([P, DIM], F32, tag=f"g{ct}", bufs=1, name=f"wp{ct}")
                for ct in range(KT)
            ]

            def stage_W(h):
                for ct in range(KT):
                    for dt2 in range(KT):
                        # WstarT[c_in, o] += sum_d wv[d, c_in] M_hT[d, o]
                        nc.tensor.matmul(
                            wst_ps[ct][:],
                            wvo(dt2)[:, OV + h * DIM + ct * P : OV + h * DIM + (ct + 1) * P],
                            m_sb[(h, dt2)][:],
                            start=(h == 0 and dt2 == 0),
                            stop=(h == HEADS - 1 and dt2 == KT - 1),
                        )

            # pipelined emission (PE order)
            stage_A(0)
            stage_L(0)
            stage_L(1)
            stage_A(1)
            stage_L(2)
            stage_L(3)
            stage_M(0)
            stage_M(1)
            stage_W(0)
            stage_M(2)
            stage_W(1)
            stage_M(3)
            stage_W(2)
            stage_W(3)

            wst_sb = []
            for ct in range(KT):
                wt = spool.tile([P, DIM], BF16, tag=f"wst{ct}", bufs=1, name=f"wt{ct}")
                if ct == 0:
                    nc.vector.tensor_copy(wt[:], wst_ps[ct][:])
                else:
                    nc.scalar.copy(wt[:], wst_ps[ct][:])
                wst_sb.append(wt)

            # ---- y = WstarT.T @ x + b  (bf16 out, tapered stores) --------
            y_sb = ypool.tile([P, KT, N], BF16, tag="y", bufs=1, name="ysb")
            # store groups: (start_chunk, n_chunks) issued after chunk j
            store_after = {j: (j, 1) for j in range(NCH)}
            ytags = ["g0", "g1", "am", "am", "l", "l"]
            ycnt = 0
            for j in range(NCH):
                for ot in range(KT):
                    ytag = ytags[(j * KT + ot) % 6]
                    py = psum.tile([P, 512], F32, tag=ytag,
                                   bufs=(1 if ytag in ("g0", "g1") else 2),
                                   name=f"py{j}_{ot}")
                    for k in range(KT):
                        nc.tensor.matmul(
                            py[:],
                            wst_sb[k][:, ts(ot, P)],
                            x_sb[:, k, ts(j, 512)],
                            start=(k == 0),
                            stop=(k == KT - 1),
                        )
                    dst = y_sb[:, ot, ts(j, 512)]
                    if ycnt % 8 < 5:
                        nc.vector.tensor_scalar_add(dst, py[:], b_sb[:, ot : ot + 1])
                    else:
                        nc.scalar.add(dst, py[:], b_sb[:, ot : ot + 1])
                    ycnt += 1
                if j in store_after:
                    j0, nj = store_after[j]
                    nc.sync.dma_start(
                        y_d[:, :, j0 * 512 : (j0 + nj) * 512],
                        y_sb[:, :, j0 * 512 : (j0 + nj) * 512],
                    )


def prep_inputs(x, w_qkv, w_out, b_out):
    """Host-side packing: per-core input dicts (numpy only)."""
    x = np.asarray(x, dtype=np.float32)
    w_qkv = np.asarray(w_qkv, dtype=np.float32)
    w_out = np.asarray(w_out, dtype=np.float32)
    b_out = np.asarray(b_out, dtype=np.float32)

    scale = float(DIM) ** -0.5
    wq = w_qkv[0 * HEADS * DIM : 1 * HEADS * DIM].reshape(HEADS, DIM, DIM)
    wk = w_qkv[1 * HEADS * DIM : 2 * HEADS * DIM].reshape(HEADS, DIM, DIM)
    wv = w_qkv[2 * HEADS * DIM : 3 * HEADS * DIM].reshape(HEADS, DIM, DIM)

    # wqT[c', h*256 + c] = wq[h, c, c'] * scale
    wqT = (np.transpose(wq, (2, 0, 1)) * scale).reshape(DIM, HEADS * DIM)
    # wkT[c', h*256 + d] = wk[h, d, c']
    wkT = np.transpose(wk, (2, 0, 1)).reshape(DIM, HEADS * DIM)
    # wvn[d, h*256 + c_in] = wv[h, d, c_in]  (natural orientation, head-concat)
    wvn = np.transpose(wv, (1, 0, 2)).reshape(DIM, HEADS * DIM)
    # woT[c, h*256 + o] = w_out[o, c*HEADS + h]
    woT = np.ascontiguousarray(
        w_out.reshape(DIM, DIM, HEADS).transpose(1, 2, 0)
    ).reshape(DIM, HEADS * DIM)

    wkTs = wkT.reshape(KT, P, HEADS * DIM)
    wqTs = wqT.reshape(KT, P, HEADS * DIM)
    wvo = np.concatenate([wvn, woT], axis=1).reshape(KT, P, 2 * HEADS * DIM)
    eye = np.eye(P, dtype=np.float32)
    # w_all: [128, 8320] = wkT0 | wkT1 | eye | wqT0 | wqT1 | wvo0 | wvo1
    w_all = np.ascontiguousarray(
        np.concatenate([wkTs[0], wkTs[1], eye, wqTs[0], wqTs[1], wvo[0], wvo[1]], axis=1)
    ).astype(NPBF16)
    b = np.ascontiguousarray(b_out.reshape(KT, P).T).astype(np.float32)

    in_maps = []
    for bi in range(B):
        xb = np.ascontiguousarray(x[bi].reshape(DIM, N)).astype(NPBF16)
        # xt group g: (p, s, a, c) = x.T[(2g+s)*1024 + a*128 + p, c]
        xt = np.ascontiguousarray(
            x[bi].reshape(DIM, N).astype(np.float32).T
            .reshape(NG, 2, NT // (2 * NG), P, DIM).transpose(0, 3, 1, 2, 4)
        ).astype(NPFP8)
        xpack = np.ascontiguousarray(xb.reshape(KT, P, N).transpose(1, 0, 2))
        m = {f"xt{g}": xt[g] for g in range(NG)}
        m.update({"w": w_all, "x": xpack, "b": b})
        in_maps.append(m)
    return in_maps


_NC_CACHE = {}


def get_program():
    if "nc" not in _NC_CACHE:
        _NC_CACHE["nc"] = build_program()
    return _NC_CACHE["nc"]


def unpack_outputs(res):
    # y_d is [128, KT, N] bf16; row c = k*128+p  ->  y[p, k, n]
    y = np.stack(
        [
            np.asarray(res.results[c]["y"]).astype(np.float32).transpose(1, 0, 2)
            for c in range(N_CORES)
        ],
        axis=0,
    )
    return y.reshape(B, DIM, H, W)


def kernel(x, w_qkv, w_out, b_out, **_unused):
    nc = get_program()
    in_maps = prep_inputs(x, w_qkv, w_out, b_out)
    res = run_bass_kernel_spmd(nc, in_maps, list(range(N_CORES)))
    return unpack_outputs(res)
